# revision 12
# baseline (speedup 1.0000x reference)
"""Multi-head causal attention on 8 Trainium2 NeuronCores.

Problem: B=4, S=2048, d_model=512, H=8 heads, d_k=64, fp32, causal,
scale = 1/sqrt(d_model) (faithful source quirk).

Sharding: 32 (batch, head-group) units -> core c handles batch c%4 and
head group c//4 (4 heads = 256 projection columns). Each core computes
q/k/v projections for its column slice, causal attention for its 4
heads, and a partial output projection (its 256 rows of Wo). The host
sums the two partials per batch and adds the output bias.

On-chip layout (all matmul operands float32r = full-rate fp32 on the PE):
  qT, kT   : (256, 2048)  d_head on partitions -> feeds scores matmuls
  scoresT  : (128 keys, q) in PSUM, one keyblock at a time
  p = exp  : ACT reads scores PSUM, writes SBUF (the softmax exp)
  v_aug    : (keys, 65) per head: v columns + ones column; the PV matmul
             then yields both the context and the softmax row sums.
  ctxT     : (256, 2048) normalized context, feeds output projection.
"""

import sys

sys.path.insert(0, "/opt/trn_rl_repo")

from contextlib import ExitStack

import numpy as np

import concourse.bass as bass
import concourse.tile as tile
from concourse import bacc, mybir
from concourse.bass_utils import run_bass_kernel_spmd

FP32 = mybir.dt.float32
FP32R = mybir.dt.float32r
FP16 = mybir.dt.float16
MM = FP16  # matmul operand dtype
MM_NP = np.float16
AF = mybir.ActivationFunctionType

B, S, DM, H = 4, 2048, 512, 8
DK = DM // H  # 64
HC = 4  # heads per core
COLS = HC * DK  # 256
P = 128
NKB = S // P  # 16 key blocks
SCALE = 1.0 / float(np.sqrt(np.float32(DM)))

_CACHED_NC = None


def _split512(w):
    """split [0, w) into chunks of <=512"""
    out = []
    lo = 0
    while lo < w:
        hi = min(lo + 512, w)
        out.append((lo, hi))
        lo = hi
    return out


def build_program():
    nc = bacc.Bacc("TRN2", target_bir_lowering=False, debug=False)

    qt_d = nc.dram_tensor("QT", [DM, S], MM, kind="ExternalInput").ap()
    kt_d = nc.dram_tensor("KT", [DM, S], MM, kind="ExternalInput").ap()
    vt_d = nc.dram_tensor("VT", [DM, S], MM, kind="ExternalInput").ap()
    wq_d = nc.dram_tensor("WQ", [DM, COLS], MM, kind="ExternalInput").ap()
    wk_d = nc.dram_tensor("WK", [DM, COLS], MM, kind="ExternalInput").ap()
    wv_d = nc.dram_tensor("WV", [DM, COLS], MM, kind="ExternalInput").ap()
    wo_d = nc.dram_tensor("WO", [COLS, DM], MM, kind="ExternalInput").ap()
    bq_d = nc.dram_tensor("BQ", [COLS], FP32, kind="ExternalInput").ap()
    bk_d = nc.dram_tensor("BK", [COLS], FP32, kind="ExternalInput").ap()
    bv_d = nc.dram_tensor("BV", [COLS], FP32, kind="ExternalInput").ap()
    out_d = nc.dram_tensor("OUT", [S, DM], MM, kind="ExternalOutput").ap()

    with tile.TileContext(nc) as tc, ExitStack() as ctx:
        const = ctx.enter_context(tc.tile_pool(name="const", bufs=1))
        persist = ctx.enter_context(tc.tile_pool(name="persist", bufs=1))

        # ---- constants ----
        wq_sb = const.tile([P, DM // P, COLS], MM, tag="wq")
        wk_sb = const.tile([P, DM // P, COLS], MM, tag="wk")
        wv_sb = const.tile([P, DM // P, COLS], MM, tag="wv")
        for w_sb, w_d in ((wq_sb, wq_d), (wk_sb, wk_d), (wv_sb, wv_d)):
            nc.sync.dma_start(
                out=w_sb[:], in_=w_d.rearrange("(t p) c -> p t c", p=P)
            )
        wo_sb = const.tile([P, COLS // P, DM], MM, tag="wo")
        nc.sync.dma_start(out=wo_sb[:], in_=wo_d.rearrange("(t p) c -> p t c", p=P))

        bq_sb = const.tile([P, COLS // P], FP32, tag="bq")
        bk_sb = const.tile([P, COLS // P], FP32, tag="bk")
        nc.sync.dma_start(out=bq_sb[:], in_=bq_d.rearrange("(c p) -> p c", p=P))
        nc.sync.dma_start(out=bk_sb[:], in_=bk_d.rearrange("(c p) -> p c", p=P))
        bv_row = const.tile([1, COLS], FP32, tag="bvrow")
        nc.sync.dma_start(out=bv_row[:], in_=bv_d.rearrange("(a c) -> a c", a=1))
        bv_b = const.tile([P, COLS], FP32, tag="bvb")
        nc.gpsimd.partition_broadcast(bv_b[:], bv_row[:])

        # multiplicative causal mask for the diagonal block, in scoresT
        # coords (partition = key, free = query): keep where q >= k.
        mask_f32 = const.tile([P, P], FP32, tag="maskf")
        nc.gpsimd.memset(mask_f32[:], 1.0)
        nc.gpsimd.affine_select(
            out=mask_f32[:],
            in_=mask_f32[:],
            compare_op=mybir.AluOpType.is_ge,
            fill=0.0,
            base=0,
            pattern=[[1, P]],
            channel_multiplier=-1,
        )
        mask_sb = const.tile([P, P], MM, tag="mask")
        nc.vector.tensor_copy(mask_sb[:], mask_f32[:])

        # ---- persistent activations ----
        v_sb = persist.tile([P, NKB, HC, DK + 1], MM, tag="vaug")
        ones_f32 = const.tile([P, NKB, HC, 1], FP32, tag="ones")
        nc.gpsimd.memset(ones_f32[:], 1.0)
        nc.vector.tensor_copy(v_sb[:, :, :, DK : DK + 1], ones_f32[:])
        ctxt_sb = [persist.tile([P, S], MM, tag=f"ctxt{i}", name=f"ctxt{i}") for i in range(2)]
        # Per-head q/k with the 64 head dims DUPLICATED onto both partition
        # halves: scores then contract over K=128, which is what the PE's
        # activity monitor needs to release the 2.4 GHz clock (K=64 streams
        # leave it throttled at 1.2 GHz). The doubled dot product is folded
        # into the exp scale.
        qt_dup = [persist.tile([P, S], MM, tag=f"qtd{h}", name=f"qtd{h}") for h in range(HC)]
        kt_dup = [persist.tile([P, S], MM, tag=f"ktd{h}", name=f"ktd{h}") for h in range(HC)]

        # ================= Phase A: projections =================
        # j (the d_model contraction tile) is the OUTER loop so the first
        # matmul starts as soon as the first 128-row slab of the input has
        # landed; 8 live PSUM accumulators (one bank each). The PSUM->SBUF
        # copybacks write each head's 64 output dims to BOTH partition
        # halves of qt_dup/kt_dup (split between VectorE and the otherwise
        # idle ScalarE): scores then contract over K=128, which the PE's
        # activity monitor needs to release the 2.4 GHz clock (K=64 streams
        # leave it throttled at 1.2 GHz). The doubled dot product is folded
        # into the exp scale.
        xin = ctx.enter_context(tc.tile_pool(name="xin", bufs=3))
        with tc.tile_pool(name="pj_psum", bufs=8, space="PSUM") as pj_psum:
            for x_d, w_sb, b_sb, dup in (
                (kt_d, wk_sb, bk_sb, kt_dup),
                (qt_d, wq_sb, bq_sb, qt_dup),
            ):
                nm = "k" if dup is kt_dup else "q"
                pss = [
                    pj_psum.tile([P, 512], FP32, tag="ps", name=f"ps{nm}{i}")
                    for i in range(8)
                ]
                for j in range(DM // P):
                    xt = xin.tile([P, S], MM, tag="x")
                    nc.sync.dma_start(out=xt[:], in_=x_d[j * P : (j + 1) * P, :])
                    for cc in range(COLS // P):
                        for t in range(S // 512):
                            nc.tensor.matmul(
                                pss[cc * 4 + t][:],
                                w_sb[:, j, cc * P : (cc + 1) * P],
                                xt[:, t * 512 : (t + 1) * 512],
                                start=(j == 0),
                                stop=(j == DM // P - 1),
                            )
                for cc in range(COLS // P):
                    for t in range(S // 512):
                        ts_ = slice(t * 512, (t + 1) * 512)
                        ps = pss[cc * 4 + t]
                        for hh in range(2):
                            h = cc * 2 + hh
                            hs = slice(hh * DK, (hh + 1) * DK)
                            nc.vector.tensor_scalar_add(
                                dup[h][0:DK, ts_], ps[hs, :], b_sb[hs, cc : cc + 1]
                            )
                            nc.scalar.activation(
                                dup[h][DK : 2 * DK, ts_],
                                ps[hs, :],
                                AF.Identity,
                                bias=b_sb[hs, cc : cc + 1],
                            )

        # ================= Phase B: attention per head =================
        # The per-head accumulator is split into two q-halves of (65, 1024)
        # = 2 PSUM banks each, so normalization of the low half (complete
        # after kb=7) overlaps with accumulation of the high half, and the
        # next head's PV can start before this head fully drains.
        with tc.tile_pool(name="pt", bufs=3) as pt_pool, tc.tile_pool(
            name="sc_psum", bufs=2, space="PSUM"
        ) as sc_psum, tc.tile_pool(
            name="out_psum", bufs=2, space="PSUM"
        ) as out_psum, tc.tile_pool(name="norm", bufs=4) as norm_pool:

            def normalize(po, half, ti, po_):
                """ctxT[head, half] = po[0:64] * (1 / po[64]) for one q-half."""
                sums = norm_pool.tile([1, 1024], FP32, tag="sums", name=f"s{ti}_{po_}_{half}")
                nc.vector.tensor_copy(sums[:], po[DK : DK + 1, :])
                recip = norm_pool.tile([1, 1024], FP32, tag="recip", name=f"r{ti}_{po_}_{half}")
                nc.vector.reciprocal_approx_fast(out=recip[:], in_=sums[:])
                bcast = norm_pool.tile([DK, 1024], FP32, tag="bcast", name=f"b{ti}_{po_}_{half}")
                nc.gpsimd.partition_broadcast(bcast[:], recip[:])
                nc.vector.tensor_mul(
                    ctxt_sb[ti][po_ : po_ + DK, half * 1024 : (half + 1) * 1024],
                    po[0:DK, :],
                    bcast[:],
                )

            # V projection, interleaved with early attention: natural
            # layout (tokens on partitions) + ones column; accumulators
            # rotate through the same 2 PSUM slots as the score tiles.
            v_tiles = []
            for j in range(DM // P):
                vt = xin.tile([P, S], MM, tag="xv", bufs=4, name=f"vt{j}")
                nc.sync.dma_start(out=vt[:], in_=vt_d[j * P : (j + 1) * P, :])
                v_tiles.append(vt)
            for tb in range(NKB):
                ps = sc_psum.tile([P, 1024], FP32, tag="sc", name=f"vps{tb}")
                for j in range(DM // P):
                    nc.tensor.matmul(
                        ps[:, :COLS],
                        v_tiles[j][:, tb * P : (tb + 1) * P],
                        wv_sb[:, j, :],
                        start=(j == 0),
                        stop=(j == DM // P - 1),
                    )
                for h in range(HC):
                    nc.vector.tensor_add(
                        v_sb[:, tb, h, 0:DK],
                        ps[:, h * DK : (h + 1) * DK],
                        bv_b[:, h * DK : (h + 1) * DK],
                    )

            for h in range(HC):
                ti, po_ = h // 2, (h % 2) * DK
                qt_h = qt_dup[h]
                kt_h = kt_dup[h]
                po_half = [
                    out_psum.tile([DK + 1, 1024], FP32, tag="po", name=f"po{h}_{half}")
                    for half in range(2)
                ]
                for kb in range(NKB):
                    q0 = kb * P
                    for jt in range(q0 // 1024, S // 1024):
                        tq0 = max(q0, 1024 * jt)
                        w = 1024 * (jt + 1) - tq0
                        sc = sc_psum.tile([P, 1024], FP32, tag="sc", name=f"sc{h}_{kb}_{jt}")
                        for lo, hi in _split512(w):
                            nc.tensor.matmul(
                                sc[:, lo:hi],
                                kt_h[:, q0 : q0 + P],
                                qt_h[:, tq0 + lo : tq0 + hi],
                                start=True,
                                stop=True,
                            )
                        pt = pt_pool.tile([P, 1024], MM, tag="pt", name=f"pt{h}_{kb}_{jt}")
                        nc.scalar.activation(
                            pt[:, :w], sc[:, :w], AF.Exp, scale=SCALE / 2.0
                        )
                        if tq0 == q0:  # tile holds the diagonal block
                            nc.vector.tensor_mul(
                                pt[:, 0:P], pt[:, 0:P], mask_sb[:]
                            )
                        # PV + row-sum accumulation for this keyblock into
                        # the jt-half accumulator (psum-bank-aligned chunks)
                        po = po_half[jt]
                        hoff = jt * 1024
                        qc = (tq0 // 512) * 512
                        last_kb_for_half = (kb == NKB - 1) if jt == 1 else (kb == 7)
                        while qc < tq0 + w:
                            glo, ghi = max(tq0, qc), min(tq0 + w, qc + 512)
                            nc.tensor.matmul(
                                po[:, glo - hoff : ghi - hoff],
                                v_sb[:, kb, h, :],
                                pt[:, glo - tq0 : ghi - tq0],
                                start=(kb == 0),
                                stop=last_kb_for_half,
                                skip_group_check=True,
                            )
                            qc += 512
                    if kb == 7:
                        normalize(po_half[0], 0, ti, po_)
                normalize(po_half[1], 1, ti, po_)

        # ================= Phase C: output projection =================
        with tc.tile_pool(name="op_psum", bufs=4, space="PSUM") as op_psum, tc.tile_pool(
            name="osb", bufs=3
        ) as osb:
            for tb in range(S // P):
                ps = op_psum.tile([P, DM], FP32, tag="ops")
                for cc in range(COLS // P):
                    nc.tensor.matmul(
                        ps[:],
                        ctxt_sb[cc][:, tb * P : (tb + 1) * P],
                        wo_sb[:, cc, :],
                        start=(cc == 0),
                        stop=(cc == COLS // P - 1),
                    )
                o = osb.tile([P, DM], MM, tag="o")
                nc.vector.tensor_copy(o[:], ps[:])
                nc.sync.dma_start(
                    out=out_d[tb * P : (tb + 1) * P, :], in_=o[:]
                )

    nc.compile()
    return nc


def _get_nc():
    global _CACHED_NC
    if _CACHED_NC is None:
        _CACHED_NC = build_program()
    return _CACHED_NC


def make_in_maps(Q, K, V, Wq, bq, Wk, bk, Wv, bv, Wo, bo):
    f32 = lambda a: np.ascontiguousarray(a, dtype=np.float32)
    mm = lambda a: np.ascontiguousarray(np.asarray(a), dtype=MM_NP)
    qt = [mm(np.asarray(Q[b]).T) for b in range(B)]
    kt = [mm(np.asarray(K[b]).T) for b in range(B)]
    vt = [mm(np.asarray(V[b]).T) for b in range(B)]
    maps = []
    for c in range(8):
        b, hg = c % B, c // B
        cs = slice(hg * COLS, (hg + 1) * COLS)
        maps.append(
            {
                "QT": qt[b],
                "KT": kt[b],
                "VT": vt[b],
                "WQ": mm(Wq[:, cs]),
                "WK": mm(Wk[:, cs]),
                "WV": mm(Wv[:, cs]),
                "WO": mm(Wo[cs, :]),
                "BQ": f32(bq[cs]),
                "BK": f32(bk[cs]),
                "BV": f32(bv[cs]),
            }
        )
    return maps


def assemble(results, bo):
    out = np.empty((B, S, DM), dtype=np.float32)
    for b in range(B):
        out[b] = results[b]["OUT"].astype(np.float32) + results[b + B][
            "OUT"
        ].astype(np.float32)
    out += np.asarray(bo, dtype=np.float32)
    return out


def kernel(Q, K, V, Wq, bq, Wk, bk, Wv, bv, Wo, bo):
    nc = _get_nc()
    maps = make_in_maps(Q, K, V, Wq, bq, Wk, bk, Wv, bv, Wo, bo)
    res = run_bass_kernel_spmd(nc, maps, list(range(8)))
    return assemble(res.results, bo)
